# revision 7
# baseline (speedup 1.0000x reference)
"""Trainium2 Bass kernel for nn_HPool histogram_binning.

Math: z[n,c] = sum_hw tanh(x) * coeff[c, bin(x)] with 32 uniform bins over
[min(x), max(x)] (global).  Rewritten via cumulative-threshold form to avoid
any gather:
    coeff[c, b] = A_c + sum_{j=1..31} d[c,j] * [b >= j]
    z[n,c]      = A_c * T[n,c] + sum_j d[c,j] * S_j[n,c]
    T[n,c]      = sum_hw tanh(x)
    S_j[n,c]    = sum_hw tanh(x) * [x >= tau_j]     (tau_j = gmin + j*range/32)
Each S_j is one fused compare+mult+reduce (scalar_tensor_tensor) pass.

Sharding: data-parallel over N across 8 cores (8 samples each).
"""

import os
import numpy as np

N, C, H, W, BINS = 64, 64, 128, 128, 32
HW = H * W
NCORES = 8
NPC = N // NCORES          # samples per core
ROWS = NPC * C             # 512 rows per core, row r = n_local*C + c
P = 128
NT = ROWS // P             # 4 row-tiles
F = 2048                   # free-dim chunk
NF = HW // F               # 8 chunks per row-tile
NCHUNK = NT * NF

LAST_EXEC_NS = None
_CACHE = {}

# Bins handled by GPSIMD (rest on DVE). Tuned after profiling.
GP_BINS = ()


def _new_nc():
    import concourse.bacc as bacc

    return bacc.Bacc(
        "TRN2", target_bir_lowering=False, debug=False, num_devices=NCORES
    )


def _build_main():
    """Main kernel: thresholds are a [P, 31] input, z is the [ROWS, 1] output."""
    import concourse.mybir as mybir
    from concourse.tile import TileContext

    fp32 = mybir.dt.float32
    AX = mybir.AxisListType.X
    OP = mybir.AluOpType

    nc = _new_nc()
    xs = nc.dram_tensor("xs", [ROWS, HW], fp32, kind="ExternalInput")
    dA = nc.dram_tensor("dA", [P, BINS], fp32, kind="ExternalInput")
    thi = nc.dram_tensor("th", [P, BINS - 1], fp32, kind="ExternalInput")
    z = nc.dram_tensor("z", [ROWS, 1], fp32, kind="ExternalOutput")

    with TileContext(nc, num_cores=NCORES) as tc:
        with (
            tc.tile_pool(name="xp", bufs=4) as xp,
            tc.tile_pool(name="tp", bufs=2) as tp,
            tc.tile_pool(name="sp", bufs=2) as sp,
            tc.tile_pool(name="stat", bufs=1) as stat,
        ):
            dAs = stat.tile([P, BINS], fp32, tag="dAs")
            nc.sync.dma_start(out=dAs[:], in_=dA[:, :])
            th = stat.tile([P, BINS - 1], fp32, tag="th")
            nc.sync.dma_start(out=th[:], in_=thi[:, :])

            for t in range(NT):
                S = sp.tile([P, (BINS - 1) * NF], fp32, tag="S")
                TA = sp.tile([P, NF], fp32, tag="TA")
                for f in range(NF):
                    X = xp.tile([P, F], fp32, tag="X")
                    nc.sync.dma_start(
                        out=X[:], in_=xs[t * P:(t + 1) * P, f * F:(f + 1) * F]
                    )
                    T = tp.tile([P, F], fp32, tag="T")
                    nc.scalar.activation(
                        out=T[:], in_=X[:],
                        func=mybir.ActivationFunctionType.Tanh,
                        accum_out=TA[:, f:f + 1],
                    )
                    SC = tp.tile([P, F], fp32, tag="SC")
                    SCG = tp.tile([P, F], fp32, tag="SCG") if GP_BINS else None
                    for j in range(1, BINS):
                        eng = nc.gpsimd if j in GP_BINS else nc.vector
                        out_t = SCG if j in GP_BINS else SC
                        eng.scalar_tensor_tensor(
                            out=out_t[:], in0=X[:], scalar=th[:, j - 1:j], in1=T[:],
                            op0=OP.is_ge, op1=OP.mult,
                            accum_out=S[:, (j - 1) * NF + f:(j - 1) * NF + f + 1],
                        )
                V = sp.tile([P, BINS], fp32, tag="V")
                nc.vector.tensor_reduce(out=V[:, 0:1], in_=TA[:], axis=AX, op=OP.add)
                for j in range(1, BINS):
                    nc.vector.tensor_reduce(
                        out=V[:, j:j + 1], in_=S[:, (j - 1) * NF:j * NF],
                        axis=AX, op=OP.add,
                    )
                ZC = sp.tile([P, BINS], fp32, tag="ZC")
                zcol = sp.tile([P, 1], fp32, tag="zcol")
                nc.vector.tensor_tensor(out=ZC[:], in0=V[:], in1=dAs[:], op=OP.mult)
                nc.vector.tensor_reduce(out=zcol[:], in_=ZC[:], axis=AX, op=OP.add)
                nc.sync.dma_start(out=z[t * P:(t + 1) * P, :], in_=zcol[:])
    nc.compile()
    return nc


def kernel(x: np.ndarray, coeff: np.ndarray) -> np.ndarray:
    global LAST_EXEC_NS
    from concourse.bass_utils import run_bass_kernel_spmd

    x = np.asarray(x, dtype=np.float32)
    coeff = np.asarray(coeff, dtype=np.float32)

    if "nc" not in _CACHE:
        _CACHE["nc"] = _build_main()
    nc = _CACHE["nc"]

    gmin = np.float32(x.min())
    gmax = np.float32(x.max())
    step = np.float32((gmax - gmin) * np.float32(1.0 / 32.0))
    js = np.arange(1, BINS, dtype=np.float32)
    taus = (gmin + js * step).astype(np.float32)        # tau_1..tau_31
    th128 = np.ascontiguousarray(np.tile(taus, (P, 1)), dtype=np.float32)

    dA64 = np.concatenate([coeff[:, :1], np.diff(coeff, axis=1)], axis=1)
    dA128 = np.ascontiguousarray(np.tile(dA64, (2, 1)), dtype=np.float32)

    xr = x.reshape(N, C, HW)
    in_maps = []
    for k in range(NCORES):
        shard = np.ascontiguousarray(
            xr[k * NPC:(k + 1) * NPC].reshape(ROWS, HW), dtype=np.float32
        )
        in_maps.append({"xs": shard, "dA": dA128, "th": th128})

    trace = bool(os.environ.get("KERNEL_TRACE"))
    res = run_bass_kernel_spmd(
        nc, in_maps, list(range(NCORES)), trace=trace,
    )
    LAST_EXEC_NS = res.exec_time_ns

    out = np.empty((N, C), dtype=np.float32)
    for k in range(NCORES):
        out[k * NPC:(k + 1) * NPC] = res.results[k]["z"].reshape(NPC, C)
    return out


# revision 13
# speedup vs baseline: 52.0161x; 52.0161x over previous
"""Trainium2 Bass kernel for nn_HPool histogram_binning.

Math: z[n,c] = sum_hw tanh(x) * coeff[c, bin(x)] with 32 uniform bins over
[min(x), max(x)] (global).  Rewritten via cumulative-threshold form to avoid
any gather:
    coeff[c, b] = A_c + sum_{j=1..31} d[c,j] * [b >= j]
    z[n,c]      = A_c * T[n,c] + sum_j d[c,j] * S_j[n,c]
    T[n,c]      = sum_hw tanh(x)
    S_j[n,c]    = sum_hw tanh(x) * [x >= tau_j]     (tau_j = gmin + j*range/32)
Each S_j is one fused compare+mult+reduce (scalar_tensor_tensor) pass.

Sharding: data-parallel over N across 8 cores (8 samples each).
"""

import os
import numpy as np

N, C, H, W, BINS = 64, 64, 128, 128, 32
HW = H * W
NCORES = 8
NPC = N // NCORES          # samples per core
ROWS = NPC * C             # 512 rows per core, row r = n_local*C + c
P = 128
NT = ROWS // P             # 4 row-tiles
F = 2048                   # free-dim chunk
NF = HW // F               # 8 chunks per row-tile
NCHUNK = NT * NF

LAST_EXEC_NS = None
_CACHE = {}
import os as _os
NBINS_ACTIVE = int(_os.environ.get("KERNEL_NBINS", str(BINS - 1)))  # STT passes to emit
FP16 = bool(int(_os.environ.get("KERNEL_FP16", "0")))

# Engine assignment per bin j in 1..31 (rest on DVE). Tuned via cost model.
GP_BINS = ()                        # Pool can't run TensorScalarPtr (walrus check)
ACT_BINS = tuple(range(1, 12))      # scalar-engine relu/sign bins
VCOLS = 64                          # V layout: 0=T, 1..31=S/R, 32..62=G, 63=const


def _new_nc():
    import concourse.bacc as bacc

    return bacc.Bacc(
        "TRN2", target_bir_lowering=False, debug=False, num_devices=NCORES
    )


def _build_main():
    """Main kernel: thresholds are a [P, 31] input, z is the [ROWS, 1] output."""
    import concourse.mybir as mybir
    from concourse.tile import TileContext

    fp32 = mybir.dt.float32
    fp16 = mybir.dt.float16
    cdt = fp16 if FP16 else fp32
    AX = mybir.AxisListType.X
    OP = mybir.AluOpType

    nc = _new_nc()
    xs = nc.dram_tensor("xs", [ROWS, HW], fp32, kind="ExternalInput")
    dA = nc.dram_tensor("dA", [P, VCOLS], fp32, kind="ExternalInput")
    thi = nc.dram_tensor("th", [P, BINS - 1], fp32, kind="ExternalInput")
    ntt = nc.dram_tensor("ntt", [P, BINS - 1], fp32, kind="ExternalInput")  # -tanh(tau)
    nth = nc.dram_tensor("nth", [P, BINS - 1], fp32, kind="ExternalInput")  # -tau
    z = nc.dram_tensor("z", [ROWS, 1], fp32, kind="ExternalOutput")

    with TileContext(nc, num_cores=NCORES) as tc:
        with (
            tc.tile_pool(name="xp", bufs=4) as xp,
            tc.tile_pool(name="tp", bufs=2) as tp,
            tc.tile_pool(name="sp", bufs=2) as sp,
            tc.tile_pool(name="stat", bufs=1) as stat,
        ):
            dAs = stat.tile([P, VCOLS], fp32, tag="dAs")
            nc.sync.dma_start(out=dAs[:], in_=dA[:, :])
            th = stat.tile([P, BINS - 1], fp32, tag="th")
            nc.sync.dma_start(out=th[:], in_=thi[:, :])
            ntts = stat.tile([P, BINS - 1], fp32, tag="ntts")
            nc.sync.dma_start(out=ntts[:], in_=ntt[:, :])
            nths = stat.tile([P, BINS - 1], fp32, tag="nths")
            nc.sync.dma_start(out=nths[:], in_=nth[:, :])
            if FP16:
                thh = stat.tile([P, BINS - 1], fp16, tag="thh")
                nc.vector.tensor_copy(out=thh[:], in_=th[:])
            else:
                thh = th

            for t in range(NT):
                S = sp.tile([P, 2 * (BINS - 1) * NF], fp32, tag="S")
                TA = sp.tile([P, NF], fp32, tag="TA")
                for f in range(NF):
                    X = xp.tile([P, F], fp32, tag="X")
                    nc.sync.dma_start(
                        out=X[:], in_=xs[t * P:(t + 1) * P, f * F:(f + 1) * F]
                    )
                    T = tp.tile([P, F], cdt, tag="T")
                    nc.scalar.activation(
                        out=T[:], in_=X[:],
                        func=mybir.ActivationFunctionType.Tanh,
                        accum_out=TA[:, f:f + 1],
                    )
                    if FP16:
                        Xh = tp.tile([P, F], fp16, tag="Xh")
                        nc.scalar.copy(out=Xh[:], in_=X[:])
                    else:
                        Xh = X
                    SC = tp.tile([P, F], cdt, tag="SC")
                    if GP_BINS:
                        SCG = tp.tile([P, F], cdt, tag="SCG")
                    else:
                        SCG = None
                    SA = tp.tile([P, F], fp32, tag="SA")
                    SB = tp.tile([P, F], fp32, tag="SB")
                    for j in range(1, NBINS_ACTIVE + 1):
                        sacc = S[:, (j - 1) * NF + f:(j - 1) * NF + f + 1]
                        if j in ACT_BINS:
                            nc.scalar.activation(
                                out=SA[:], in_=T[:],
                                func=mybir.ActivationFunctionType.Relu,
                                bias=ntts[:, j - 1:j], accum_out=sacc,
                            )
                            gacc = S[:, ((BINS - 1) + (j - 1)) * NF + f:
                                     ((BINS - 1) + (j - 1)) * NF + f + 1]
                            nc.scalar.activation(
                                out=SB[:], in_=X[:],
                                func=mybir.ActivationFunctionType.Sign,
                                bias=nths[:, j - 1:j], accum_out=gacc,
                            )
                            continue
                        eng = nc.gpsimd if j in GP_BINS else nc.vector
                        out_t = SCG if j in GP_BINS else SC
                        eng.scalar_tensor_tensor(
                            out=out_t[:], in0=Xh[:], scalar=thh[:, j - 1:j], in1=T[:],
                            op0=OP.is_ge, op1=OP.mult,
                            accum_out=sacc,
                        )
                V = sp.tile([P, VCOLS], fp32, tag="V")
                nc.vector.memset(V[:], 0.0)
                nc.vector.tensor_reduce(out=V[:, 0:1], in_=TA[:], axis=AX, op=OP.add)
                for j in range(1, NBINS_ACTIVE + 1):
                    nc.vector.tensor_reduce(
                        out=V[:, j:j + 1], in_=S[:, (j - 1) * NF:j * NF],
                        axis=AX, op=OP.add,
                    )
                    if j in ACT_BINS:
                        nc.vector.tensor_reduce(
                            out=V[:, 31 + j:32 + j],
                            in_=S[:, ((BINS - 1) + (j - 1)) * NF:
                                   ((BINS - 1) + j) * NF],
                            axis=AX, op=OP.add,
                        )
                nc.vector.memset(V[:, 63:64], 1.0)
                ZC = sp.tile([P, VCOLS], fp32, tag="ZC")
                zcol = sp.tile([P, 1], fp32, tag="zcol")
                nc.vector.tensor_tensor(out=ZC[:], in0=V[:], in1=dAs[:], op=OP.mult)
                nc.vector.tensor_reduce(out=zcol[:], in_=ZC[:], axis=AX, op=OP.add)
                nc.sync.dma_start(out=z[t * P:(t + 1) * P, :], in_=zcol[:])
    nc.compile()
    return nc


def _prep_in_maps(x: np.ndarray, coeff: np.ndarray):
    gmin = np.float32(x.min())
    gmax = np.float32(x.max())
    step = np.float32((gmax - gmin) * np.float32(1.0 / 32.0))
    js = np.arange(1, BINS, dtype=np.float32)
    taus = (gmin + js * step).astype(np.float32)        # tau_1..tau_31
    th128 = np.ascontiguousarray(np.tile(taus, (P, 1)), dtype=np.float32)

    tanh_tau = np.tanh(taus.astype(np.float64)).astype(np.float32)
    ntt128 = np.ascontiguousarray(np.tile(-tanh_tau, (P, 1)), dtype=np.float32)
    nth128 = np.ascontiguousarray(np.tile(-taus, (P, 1)), dtype=np.float32)

    d64 = np.diff(coeff, axis=1)                     # d_j, j=1..31  [64,31]
    W64 = np.zeros((C, VCOLS), dtype=np.float64)
    W64[:, 0] = coeff[:, 0]                          # A_c * T
    W64[:, 1:32] = d64                               # d_j * (S_j or R_j)
    const = np.zeros(C, dtype=np.float64)
    for j in ACT_BINS:
        tt = np.float64(tanh_tau[j - 1])
        W64[:, 32 + j - 1] = d64[:, j - 1] * tt / 2.0      # d_j*tt*G_j/2
        const += d64[:, j - 1] * tt * (HW / 2.0)           # d_j*tt*HW/2
    W64[:, 63] = const
    dA128 = np.ascontiguousarray(np.tile(W64.astype(np.float32), (2, 1)))

    xr = x.reshape(N, C, HW)
    in_maps = []
    for k in range(NCORES):
        shard = np.ascontiguousarray(
            xr[k * NPC:(k + 1) * NPC].reshape(ROWS, HW), dtype=np.float32
        )
        in_maps.append({"xs": shard, "dA": dA128, "th": th128,
                        "ntt": ntt128, "nth": nth128})
    return in_maps


def kernel(x: np.ndarray, coeff: np.ndarray) -> np.ndarray:
    global LAST_EXEC_NS
    from concourse.bass_utils import run_bass_kernel_spmd

    x = np.asarray(x, dtype=np.float32)
    coeff = np.asarray(coeff, dtype=np.float32)

    if "nc" not in _CACHE:
        _CACHE["nc"] = _build_main()
    nc = _CACHE["nc"]

    in_maps = _prep_in_maps(x, coeff)

    trace = bool(os.environ.get("KERNEL_TRACE"))
    res = run_bass_kernel_spmd(
        nc, in_maps, list(range(NCORES)), trace=trace,
    )
    LAST_EXEC_NS = res.exec_time_ns

    out = np.empty((N, C), dtype=np.float32)
    for k in range(NCORES):
        out[k * NPC:(k + 1) * NPC] = res.results[k]["z"].reshape(NPC, C)
    return out
